# revision 2
# baseline (speedup 1.0000x reference)
"""CBAM3D Trainium2 kernel: 8-core SPMD, D-sharded, bf16 compute path.

x [2, 64, 64, 64, 64] f32. Each core owns an 8-plane D-slab, SBUF-resident as
bf16 [(b,c)=128 partitions, d*hw free] (halves SBUF + 1 cyc/row PE streaming
at any N + FWL weight loads; rel err ~3e-3, gate is 2e-2).

Phases per core:
 1. One 2 MiB DMA per plane (alternating qSP/qAct HWDGE rings); ScalarE
    Copy+accum_out casts the bf16 slab and accumulates the spatial sum; DVE
    reduce_max on the f32 staging tile. 1 KB stats AllGather; dummy warm
    matmuls keep the PE clock at 2.4 GHz through the collective; MLP -> ca.
 2. Channel-mean first (block-diag PE matmuls, all 8 planes) -> cmn_sb; then
    per-plane channel-max: PE transposes x chunks against diag(ca) (bf16),
    DVE strided reduce_max -> catmax. Boundary planes (0-2, 5-7) first; their
    cat rows go to DRAM and a 192 KiB bf16 boundary AllGather fires while the
    interior planes (3,4) still compute.
 3. Window assembly without select matmuls: local cat scatters into the
    padded (70x72-pitch) window by SBUF->SBUF DMA (partition-crossing); the
    6 halo planes come from the gathered boundary buffer via a per-core
    one-hot PE gather matmul, then scatter-DMA into the window.
 4. 7x7x7 conv as 28 shifted-AP bf16 PE matmuls per 512-col chunk; conv
    chains interleave ahead of the sigmoid-dependent gate matmuls so the PE
    never stalls on ScalarE; DVE multiplies x straight out of gate PSUM into
    an fp32 stage; stores own the sync HWDGE ring so the next rep's loads
    (scalar+gpsimd rings) are never queued behind them.
Self-contained: hardcodes shapes/sharding; inputs are repacked host-side.
"""
import numpy as np

import concourse.bass as bass
import concourse.mybir as mybir
import concourse.tile as tile
import concourse.bacc as bacc
import concourse.tile_utils as tile_utils

tile_utils.max_sbuf_usage = 204 * 1024

F32 = mybir.dt.float32
BF16 = mybir.dt.bfloat16
ALU = mybir.AluOpType
ACTF = mybir.ActivationFunctionType

NCORES = 8
B, C, D, H, W = 2, 64, 64, 64, 64
HW = H * W                      # 4096
DL = D // NCORES                # 8 planes per core
R = C // 8                      # 8 reduced channels
KS, PAD = 7, 3
DE = DL + 2 * PAD               # 14 extended planes per core window
NTAPS = KS * KS
NPAIR = KS * 4                  # 28 paired taps (2 w-taps each)
NCHUNK = HW // 512              # 8 chunks of 512 per plane
WP = 72                         # padded window w-pitch (4B-aligned bf16 rows)
BND = [0, 1, 2, 5, 6, 7]        # boundary planes (cat_bnd order)
INT = [3, 4]                    # interior planes

_CACHED = {}


def _build_nc(timing=False, reps=1):
    ndev = 1 if timing else NCORES
    nc = bacc.Bacc("TRN2", target_bir_lowering=False, debug=False, num_devices=ndev)

    # ---- I/O ----
    xin = nc.dram_tensor("xin", [B, C, DL, H, W], F32, kind="ExternalInput")
    w1blk = nc.dram_tensor("w1blk", [128, 16], F32, kind="ExternalInput")
    w2blk = nc.dram_tensor("w2blk", [16, 128], F32, kind="ExternalInput")
    wconv = nc.dram_tensor("wconv", [120, NPAIR * 16], BF16, kind="ExternalInput")
    cmaskdr = nc.dram_tensor("cmask", [128, 128], BF16, kind="ExternalInput")
    identdr = nc.dram_tensor("ident", [128, 128], F32, kind="ExternalInput")
    bseldr = nc.dram_tensor("bsel", [16, 8 * 128], BF16, kind="ExternalInput")
    selhdr = nc.dram_tensor("selh", [128, 2 * 24], BF16, kind="ExternalInput")
    y = nc.dram_tensor("y", [B, C, DL, H, W], F32, kind="ExternalOutput")

    # DRAM scratch for collectives
    st_dram = nc.dram_tensor("st_dram", [128, 2], F32)
    st_gath = nc.dram_tensor("st_gath", [NCORES, 128, 2], F32, addr_space="Shared")
    # boundary cat: [stat, b, {0,1,2,5,6,7}, hw] bf16 (one combined AllGather:
    # two split AGs measured slower -- they serialize on the TOPSP stream)
    cat_bnd = nc.dram_tensor("cat_bnd", [2, B, 6, HW], BF16)
    cat_bnd_g = nc.dram_tensor(
        "cat_bnd_g", [NCORES, 2, B, 6, HW], BF16, addr_space="Shared"
    )
    # local transposed channel-max bounce (partition-crossing scatter hop)
    catml = nc.dram_tensor("catml", [B, DL, HW], BF16)

    xv = xin[:].rearrange("b c d h w -> (b c) d (h w)")   # [128, 8, 4096]
    yv = y[:].rearrange("b c d h w -> (b c) d (h w)")
    cbg_rows = cat_bnd_g[:].rearrange("n s b p f -> (n s b p) f")  # [192, 4096]

    with tile.TileContext(nc, num_cores=NCORES) as tc:
        with (
            tc.tile_pool(name="persist", bufs=1) as pp,
            tc.tile_pool(name="dma_w", bufs=1) as pw,
        ):
            # ---- persistent SBUF ----
            x_sb = pp.tile([128, DL, HW], BF16)           # 64 KiB/part
            win_sb = pp.tile([128, H + 2 * PAD, WP], BF16)  # 70x72 padded window
            w1_sb = pw.tile([128, 16], F32)
            w2_sb = pw.tile([16, 128], F32)
            wc_sb = pw.tile([120, NPAIR * 16], BF16)
            cm_sb = pw.tile([128, 128], BF16)
            id_sb = pw.tile([128, 128], F32)
            idb_sb = pw.tile([128, 128], BF16)
            bsel_sb = pw.tile([16, 8 * 128], BF16)
            selh_sb = pw.tile([128, 2 * 24], BF16)
            ones16 = pw.tile([1, 16], F32)
            nc.sync.dma_start(w1_sb[:], w1blk[:, :])
            nc.sync.dma_start(w2_sb[:], w2blk[:, :])
            nc.sync.dma_start(wc_sb[:], wconv[:, :])
            nc.sync.dma_start(cm_sb[:], cmaskdr[:, :])
            nc.sync.dma_start(id_sb[:], identdr[:, :])
            nc.sync.dma_start(bsel_sb[:], bseldr[:, :])
            nc.sync.dma_start(selh_sb[:], selhdr[:, :])
            nc.gpsimd.memset(ones16[:], 1.0)
            nc.vector.tensor_copy(idb_sb[:], id_sb[:])

            # ca-folded operands, rebuilt once ca is known
            dca_sb = pw.tile([128, 128], BF16)
            cmca_sb = pw.tile([128, 128], BF16)
            bselca_sb = pw.tile([16, 8 * 128], BF16)
            car16 = pw.tile([16, 128], F32)

            stat_sum = pp.tile([128, DL], F32)
            stat_max = pp.tile([128, DL], F32)
            ca_col = pp.tile([128, 1], F32)
            catmax = pp.tile([128, B, DL, 32], BF16)      # (b, d, t) per s-lane
            cmn_sb = pp.tile([16, HW], BF16)
            sig_sb = pp.tile([16, HW], BF16)
            halo_sb = pp.tile([24, HW], BF16)

            cmx_flat = catmax[:].rearrange("p b d t -> p (b d t)")  # [128, 512]

            for _rep in range(reps):
                # ================= phase 1: load x + channel-attn stats =====
                with tc.tile_pool(name="p1io", bufs=4) as p1p:
                    for d in range(DL):
                        xl = p1p.tile([128, HW], F32, tag="xl")
                        # loads stay off the sync ring: the previous rep's
                        # y-stores drain there and HWDGE queues are FIFO
                        nc.scalar.dma_start(xl[:, 0:2048], xv[:, d, 0:2048])
                        nc.gpsimd.dma_start(xl[:, 2048:4096], xv[:, d, 2048:4096])
                        nc.scalar.activation(
                            x_sb[:, d, :], xl[:],
                            ACTF.Copy, accum_out=stat_sum[:, d : d + 1],
                        )
                        nc.vector.tensor_reduce(
                            stat_max[:, d : d + 1], xl[:],
                            axis=mybir.AxisListType.X, op=ALU.max,
                        )
                    st2 = p1p.tile([128, 2], F32, tag="st2")
                    nc.vector.tensor_reduce(
                        st2[:, 0:1], stat_sum[:], axis=mybir.AxisListType.X, op=ALU.add
                    )
                    nc.vector.tensor_reduce(
                        st2[:, 1:2], stat_max[:], axis=mybir.AxisListType.X, op=ALU.max
                    )
                    nc.sync.dma_start(st_dram[:, :], st2[:])

                if not timing:
                    nc.gpsimd.collective_compute(
                        "AllGather", ALU.bypass,
                        replica_groups=[list(range(NCORES))],
                        ins=[st_dram[:].opt()], outs=[st_gath[:].opt()],
                    )
                # zero the padded window during the collective (gpsimd FIFO
                # runs this right after the trigger, during the wait)
                nc.gpsimd.memset(win_sb[:], 0.0)

                # ======== phase 2: warm PE through the collective; MLP -> ca
                with (
                    tc.tile_pool(name="mlpsb", bufs=1) as mp,
                    tc.tile_pool(name="mlpps", bufs=1, space="PSUM") as mpp,
                    tc.tile_pool(name="warmps", bufs=1, space="PSUM") as wps_pool,
                ):
                    # dummy matmuls: keep the PE activity monitor busy during
                    # the AllGather so phase 3 starts at 2.4 GHz.  They read
                    # plane 7 (the last to land) so they cannot run early.
                    warm = wps_pool.tile([128, 512], F32, tag="warm")
                    for wi in range(76):
                        nc.tensor.matmul(
                            warm[:], cm_sb[:],
                            x_sb[:, DL - 1, (wi % 8) * 512 : (wi % 8) * 512 + 512],
                            start=True, stop=True,
                        )

                    gst = mp.tile([128, NCORES, 2], F32)
                    nc.sync.dma_start(gst[:], st_gath[:].rearrange("n p s -> p n s"))
                    avg_col = mp.tile([128, 1], F32)
                    gmax_col = mp.tile([128, 1], F32)
                    nc.vector.tensor_reduce(
                        avg_col[:], gst[:].rearrange("p n s -> p s n")[:, 0:1, :],
                        axis=mybir.AxisListType.X, op=ALU.add,
                    )
                    nc.vector.tensor_reduce(
                        gmax_col[:], gst[:].rearrange("p n s -> p s n")[:, 1:2, :],
                        axis=mybir.AxisListType.X, op=ALU.max,
                    )
                    nc.scalar.mul(avg_col[:], avg_col[:], 1.0 / float(D * HW))

                    ps1 = mpp.tile([16, 2], F32)
                    nc.tensor.matmul(ps1[:, 0:1], w1_sb[:], avg_col[:], start=True, stop=True)
                    nc.tensor.matmul(ps1[:, 1:2], w1_sb[:], gmax_col[:], start=True, stop=True)
                    r_sb = mp.tile([16, 2], F32)
                    nc.scalar.activation(r_sb[:], ps1[:], ACTF.Relu)
                    ps2 = mpp.tile([128, 2], F32)
                    nc.tensor.matmul(ps2[:], w2_sb[:], r_sb[:], start=True, stop=True)
                    z2_sb = mp.tile([128, 2], F32)
                    nc.scalar.copy(z2_sb[:], ps2[:])
                    z_sb = mp.tile([128, 1], F32)
                    nc.vector.tensor_add(z_sb[:], z2_sb[:, 0:1], z2_sb[:, 1:2])
                    nc.scalar.activation(ca_col[:], z_sb[:], ACTF.Sigmoid)

                    # fold ca into the bf16 PE-side operands
                    nc.vector.tensor_scalar(
                        dca_sb[:], idb_sb[:], ca_col[:], None, op0=ALU.mult
                    )
                    nc.vector.tensor_scalar(
                        cmca_sb[:], cm_sb[:], ca_col[:], None, op0=ALU.mult
                    )
                    pcar = mpp.tile([1, 128], F32, tag="pcar")
                    nc.tensor.matmul(pcar[:], ca_col[:], id_sb[:], start=True, stop=True)
                    car1 = mp.tile([1, 128], F32)
                    nc.scalar.copy(car1[:], pcar[:])
                    pcar16 = mpp.tile([16, 128], F32, tag="pcar16")
                    nc.tensor.matmul(pcar16[:], ones16[:], car1[:], start=True, stop=True)
                    nc.scalar.copy(car16[:], pcar16[:])
                    for db in range(DL):
                        nc.vector.tensor_tensor(
                            bselca_sb[:, db * 128 : (db + 1) * 128],
                            bsel_sb[:, db * 128 : (db + 1) * 128],
                            car16[:],
                            op=ALU.mult,
                        )

                # ===== phase 3: channel mean + max, boundary-first, AG overlap
                with (
                    tc.tile_pool(name="meanps", bufs=2, space="PSUM") as mnp,
                    tc.tile_pool(name="xctps", bufs=3, space="PSUM") as xcp,
                    tc.tile_pool(name="trps", bufs=2, space="PSUM") as trp,
                    tc.tile_pool(name="trsb", bufs=2) as trs,
                ):
                    def xct_tg(d, tg):
                        # channel-max chunk group: transpose vs diag(ca), reduce
                        pt = xcp.tile([128, 512], F32, tag="xct")
                        for j in range(4):
                            t = tg * 4 + j
                            nc.tensor.matmul(
                                pt[:, j * 128 : (j + 1) * 128],
                                x_sb[:, d, t * 128 : (t + 1) * 128],
                                dca_sb[:],
                                start=True, stop=True,
                            )
                        nc.vector.tensor_reduce(
                            catmax[:, :, d, tg * 4 : (tg + 1) * 4].rearrange(
                                "p b t -> p t b"
                            ),
                            pt[:].rearrange("p (j b c) -> p j b c", j=4, b=2),
                            axis=mybir.AxisListType.X, op=ALU.max,
                        )

                    def xct_plane(d):
                        for tg in range(8):
                            xct_tg(d, tg)

                    # channel-mean matmuls, fed one-or-two at a time between
                    # tg groups so the in-order PE FIFO fills DVE-paced gaps
                    mean_ops = []
                    mean_state = {}
                    for hq in range(8):
                        for dd in range(DL):
                            mean_ops.append(("mm", hq, dd))
                        mean_ops.append(("copy", hq, 0))

                    def mean_feed(k):
                        while k > 0 and mean_ops:
                            op, hq, dd = mean_ops.pop(0)
                            if op == "mm":
                                if dd == 0:
                                    mean_state[hq] = mnp.tile(
                                        [16, 512], F32, tag="mn", name="mnps"
                                    )
                                nc.tensor.matmul(
                                    mean_state[hq][:],
                                    cmca_sb[:, dd * 16 : (dd + 1) * 16],
                                    x_sb[:, dd, hq * 512 : (hq + 1) * 512],
                                    start=(dd == 0), stop=(dd == DL - 1),
                                )
                            else:
                                nc.scalar.copy(
                                    cmn_sb[:, hq * 512 : (hq + 1) * 512],
                                    mean_state.pop(hq)[:],
                                )
                            k -= 1

                    # boundary planes first so the halo AllGather fires early;
                    # 1.5 mean ops per tg keeps PE work/tg under the 660ns
                    # DVE reduce so the reduce pipeline never starves
                    for i, d in enumerate(BND):
                        for tg in range(8):
                            xct_tg(d, tg)
                            mean_feed(2 if (i * 8 + tg) % 2 else 1)
                    mean_feed(len(mean_ops))

                    # boundary cat rows -> cat_bnd DRAM, then the halo AllGather
                    for b in range(B):
                        nc.sync.dma_start(
                            cat_bnd[0, b, 0:3, :], cmn_sb[b * 8 : b * 8 + 3, :]
                        )
                        nc.sync.dma_start(
                            cat_bnd[0, b, 3:6, :], cmn_sb[b * 8 + 5 : b * 8 + 8, :]
                        )
                        for side, (c0, c1) in enumerate(
                            [(b * 256, b * 256 + 96), (b * 256 + 160, b * 256 + 256)]
                        ):
                            ptb = trp.tile([96, 128], BF16, tag="tr")
                            nc.tensor.transpose(
                                ptb[:], cmx_flat[:, c0:c1], idb_sb[:]
                            )
                            tsb = trs.tile([96, 128], BF16, tag="trb")
                            nc.scalar.copy(tsb[:], ptb[:])
                            nc.sync.dma_start(
                                cat_bnd[1, b, side * 3 : side * 3 + 3, :], tsb[:]
                            )
                    if not timing:
                        nc.gpsimd.collective_compute(
                            "AllGather", ALU.bypass,
                            replica_groups=[list(range(NCORES))],
                            ins=[cat_bnd[:].opt()], outs=[cat_bnd_g[:].opt()],
                        )

                    # interior planes overlap the collective
                    for d in INT:
                        xct_plane(d)

                    # --- local window fill: win rows per (half, b) block are
                    # [0:8] mean de3-10, [8:16] max de3-10, [16:28] halo
                    for b in range(B):
                        for hi, (half, wo) in enumerate(((0, PAD), (1, PAD - 1))):
                            hb = half * 64 + b * 28
                            eng = nc.scalar if hi == 0 else nc.sync
                            eng.dma_start(
                                win_sb[hb : hb + 8, PAD : PAD + H, wo : wo + W],
                                cmn_sb[b * 8 : b * 8 + 8, :],
                            )
                        for dh_half in range(2):
                            ptr = trp.tile([128, 128], BF16, tag="tr")
                            j = b * 2 + dh_half
                            nc.tensor.transpose(
                                ptr[:], cmx_flat[:, j * 128 : (j + 1) * 128], idb_sb[:]
                            )
                            tsb = trs.tile([128, 128], BF16, tag="trb")
                            nc.scalar.copy(tsb[:], ptr[:])
                            # partition-crossing hop through DRAM: tsb rows are
                            # (d_local, t); catml rows are plane-major
                            nc.sync.dma_start(
                                catml[:]
                                .rearrange("b d f -> (b d) f")
                                .rearrange("r (t p) -> (r t) p", t=32)[
                                    j * 128 : (j + 1) * 128, :
                                ],
                                tsb[:],
                            )
                        for hi, (half, wo) in enumerate(((0, PAD), (1, PAD - 1))):
                            hb = half * 64 + b * 28
                            eng = nc.scalar if hi == 0 else nc.sync
                            eng.dma_start(
                                win_sb[
                                    hb + 8 : hb + 16, PAD : PAD + H, wo : wo + W
                                ],
                                catml[b, :, :],
                            )

                # ===== phase 3b: halo gather (one-hot PE select) + window fill
                with (
                    tc.tile_pool(name="catg", bufs=3) as cgp,
                    tc.tile_pool(name="halops", bufs=2, space="PSUM") as hpp,
                    tc.tile_pool(name="warmps2", bufs=1, space="PSUM") as wp2,
                ):
                    cg0 = cgp.tile([128, HW], BF16, tag="cg")
                    cg1 = cgp.tile([64, HW], BF16, tag="cg1")
                    nc.scalar.dma_start(cg0[:], cbg_rows[0:128, :])
                    nc.scalar.dma_start(cg1[:], cbg_rows[128:192, :])
                    # keep the PE clock warm while waiting for the gather
                    warm2 = wp2.tile([128, 512], F32, tag="warm2")
                    for wi in range(24):
                        nc.tensor.matmul(
                            warm2[:], cm_sb[:],
                            x_sb[:, wi % DL, (wi % 8) * 512 : (wi % 8) * 512 + 512],
                            start=True, stop=True,
                        )
                    for ch in range(8):
                        hp = hpp.tile([24, 512], F32, tag="hp")
                        nc.tensor.matmul(
                            hp[:],
                            selh_sb[:, 0:24],
                            cg0[:, ch * 512 : (ch + 1) * 512],
                            start=True, stop=False,
                        )
                        nc.tensor.matmul(
                            hp[:],
                            selh_sb[0:64, 24:48],
                            cg1[:, ch * 512 : (ch + 1) * 512],
                            start=False, stop=True,
                        )
                        nc.scalar.copy(
                            halo_sb[:, ch * 512 : (ch + 1) * 512], hp[:]
                        )
                    # one contiguous 12-row halo fill per (half, b)
                    for b in range(B):
                        for hi, (half, wo) in enumerate(((0, PAD), (1, PAD - 1))):
                            hb = half * 64 + b * 28
                            hs = b * 12
                            eng = nc.scalar if hi == 0 else nc.sync
                            eng.dma_start(
                                win_sb[
                                    hb + 16 : hb + 28, PAD : PAD + H, wo : wo + W
                                ],
                                halo_sb[hs : hs + 12, :],
                            )

                # ===== phase 4/5: conv + sigmoid + gate + multiply + store ====
                # program order interleaves conv chains ahead of the gate
                # matmuls so the PE FIFO never stalls on the sigmoid.
                with (
                    tc.tile_pool(name="convps", bufs=2, space="PSUM") as cpp,
                    tc.tile_pool(name="gateps", bufs=2, space="PSUM") as gpp,
                    tc.tile_pool(name="stage", bufs=3) as stp,
                ):
                    def conv_chunk(ch):
                        cps = cpp.tile([16, 512], F32, tag="cps")
                        h0 = ch * 8
                        for t in range(NPAIR):
                            dh, k2 = t // 4, t % 4
                            nc.tensor.matmul(
                                cps[:],
                                wc_sb[:, t * 16 : (t + 1) * 16],
                                win_sb[
                                    0:120,
                                    h0 + dh : h0 + dh + 8,
                                    2 * k2 : 2 * k2 + W,
                                ],
                                start=(t == 0), stop=(t == NPAIR - 1),
                            )
                        nc.scalar.activation(
                            sig_sb[:, ch * 512 : (ch + 1) * 512], cps[:],
                            ACTF.Sigmoid,
                        )

                    def gate_chunkpair(chp):
                        for d in range(DL):
                            gp = gpp.tile([128, 1024], F32, tag="gate")
                            for j in range(2):
                                nc.tensor.matmul(
                                    gp[:, j * 512 : (j + 1) * 512],
                                    bselca_sb[:, d * 128 : (d + 1) * 128],
                                    sig_sb[
                                        :,
                                        (chp * 2 + j) * 512 : (chp * 2 + j + 1) * 512,
                                    ],
                                    start=True, stop=True,
                                )
                            # multiply straight out of PSUM (1x mode, but
                            # skips the ScalarE bounce) into an fp32 stage
                            ostage = stp.tile([128, 1024], F32, tag="stage")
                            nc.vector.tensor_tensor(
                                ostage[:],
                                x_sb[:, d, chp * 1024 : (chp + 1) * 1024],
                                gp[:],
                                op=ALU.mult,
                            )
                            # stores own the sync ring; the next rep's loads
                            # ride scalar+gpsimd so they are not queued behind
                            nc.sync.dma_start(
                                yv[:, d, chp * 1024 : (chp + 1) * 1024], ostage[:]
                            )

                    conv_chunk(0)
                    conv_chunk(1)
                    gate_chunkpair(0)
                    conv_chunk(2)
                    conv_chunk(3)
                    gate_chunkpair(1)
                    conv_chunk(4)
                    conv_chunk(5)
                    gate_chunkpair(2)
                    conv_chunk(6)
                    conv_chunk(7)
                    gate_chunkpair(3)

    nc.compile()
    return nc


def _wrow(s, de):
    """Window row within a (half, b) 28-row block.

    [0:8] mean de3-10, [8:16] max de3-10, [16:19] mean de0-2,
    [19:22] mean de11-13, [22:25] max de0-2, [25:28] max de11-13.
    """
    if 3 <= de <= 10:
        return s * 8 + (de - 3)
    if de < 3:
        return 16 + s * 6 + de
    return 16 + s * 6 + 3 + (de - 11)


def _host_inputs(w1, w2, w_sp):
    """Core-independent prepped weights."""
    w1blk = np.zeros((128, 16), np.float32)
    w2blk = np.zeros((16, 128), np.float32)
    for b in range(B):
        w1blk[b * 64 : (b + 1) * 64, b * 8 : (b + 1) * 8] = w1.T
        w2blk[b * 8 : (b + 1) * 8, b * 64 : (b + 1) * 64] = w2.T

    # paired taps: t = dh*4 + k2 covers (dh, 2*k2) in rows 0-55 and
    # (dh, 2*k2+1) in rows 64-119 (the w-shifted window copy)
    wconv = np.zeros((120, NPAIR, 16), np.float32)
    for t in range(NPAIR):
        dh, k2 = t // 4, t % 4
        for half, dw in ((0, 2 * k2), (1, 2 * k2 + 1)):
            if dw >= KS:
                continue
            for b in range(B):
                for s in range(2):
                    for de in range(DE):
                        for do in range(DL):
                            dd = de - do
                            if 0 <= dd < KS:
                                wconv[
                                    half * 64 + b * 28 + _wrow(s, de), t, b * 8 + do
                                ] = w_sp[0, s, dd, dh, dw]
    wconv = wconv.reshape(120, NPAIR * 16)

    cmask = np.zeros((128, 8, 16), np.float32)
    for p in range(128):
        b = p // 64
        for d in range(8):
            cmask[p, d, b * 8 + d] = 1.0 / 64.0
    cmask = cmask.reshape(128, 128)

    ident = np.eye(128, dtype=np.float32)

    bsel = np.zeros((16, 8, 128), np.float32)
    for b in range(B):
        for d in range(8):
            bsel[b * 8 + d, d, b * 64 : (b + 1) * 64] = 1.0
    bsel = bsel.reshape(16, 8 * 128)
    return (
        w1blk, w2blk,
        wconv.astype(np.float32), cmask, ident, bsel,
    )


def _selh_for_core(core):
    """One-hot gather: cat_bnd_g rows -> halo_sb rows, per core.

    cat_bnd_g flat row r = core*24 + (s*2+b)*6 + p  (p indexes planes
    {0,1,2,5,6,7}).  halo_sb row = b*12 + block*3 + k with block
    0=prev-s0, 1=next-s0, 2=prev-s1, 3=next-s1.
    """
    prev, nxt = core - 1, core + 1
    sel = np.zeros((192, 24), np.float32)
    for b in range(B):
        for s in range(2):
            for k in range(3):
                if prev >= 0:  # volume boundary: halo stays zero
                    r = prev * 24 + (s * 2 + b) * 6 + (3 + k)
                    sel[r, b * 12 + (2 * s + 0) * 3 + k] = 1.0
                if nxt < NCORES:
                    r = nxt * 24 + (s * 2 + b) * 6 + (0 + k)
                    sel[r, b * 12 + (2 * s + 1) * 3 + k] = 1.0
    # -> [128, 2*24]: col block 0 = rows 0:128, block 1 = rows 128:192 (padded)
    out = np.zeros((128, 48), np.float32)
    out[:, 0:24] = sel[0:128]
    out[0:64, 24:48] = sel[128:192]
    return out


def _get_runner(reps=1):
    """Build the SPMD executable once; return a cached callable."""
    key = ("runner", reps)
    if key in _CACHED:
        return _CACHED[key]
    import jax
    import concourse.mybir as _mybir
    from jax.experimental.shard_map import shard_map
    from jax.sharding import Mesh, PartitionSpec
    from concourse.bass2jax import (
        _bass_exec_p, install_neuronx_cc_hook, partition_id_tensor,
    )

    install_neuronx_cc_hook()
    nc = _build_nc(reps=reps)

    partition_name = (
        nc.partition_id_tensor.name if nc.partition_id_tensor else None
    )
    in_names, out_names, out_avals, zero_outs = [], [], [], []
    for alloc in nc.m.functions[0].allocations:
        if not isinstance(alloc, _mybir.MemoryLocationSet):
            continue
        name = alloc.memorylocations[0].name
        if alloc.kind == "ExternalInput":
            if name != partition_name:
                in_names.append(name)
        elif alloc.kind == "ExternalOutput":
            shape = tuple(alloc.tensor_shape)
            dtype = _mybir.dt.np(alloc.dtype)
            out_names.append(name)
            out_avals.append(jax.core.ShapedArray(shape, dtype))
            zero_outs.append(np.zeros(shape, dtype))
    n_params = len(in_names)
    all_names = tuple(in_names + out_names)
    if partition_name is not None:
        all_names = all_names + (partition_name,)

    def _exec(operands):
        if partition_name is not None:
            operands = list(operands) + [partition_id_tensor()]
        return _bass_exec_p.bind(
            *operands,
            out_avals=tuple(out_avals),
            in_names=all_names,
            out_names=tuple(out_names),
            lowering_input_output_aliases=(),
            sim_require_finite=True,
            sim_require_nnan=True,
            nc=nc,
        )

    def _body(*args):
        ins = list(args[:n_params])
        outs = list(args[n_params:])
        return tuple(_exec(ins + outs))

    devices = jax.devices()[:NCORES]
    mesh = Mesh(np.asarray(devices), ("core",))
    nin = n_params + len(out_names)
    jitted = jax.jit(
        shard_map(
            _body, mesh=mesh,
            in_specs=(PartitionSpec("core"),) * nin,
            out_specs=(PartitionSpec("core"),) * len(out_names),
            check_rep=False,
        ),
        donate_argnums=tuple(range(n_params, nin)),
        keep_unused=True,
    )

    def _concat_params(in_maps):
        per_core = [[np.asarray(m[name]) for name in in_names] for m in in_maps]
        return [
            np.concatenate([per_core[c][i] for c in range(NCORES)], axis=0)
            for i in range(n_params)
        ]

    def runner(in_maps):
        concat_in = _concat_params(in_maps) + [
            np.concatenate([z] * NCORES, axis=0) for z in zero_outs
        ]
        out_arrs = jitted(*concat_in)
        out_arrs = [np.asarray(a) for a in out_arrs]
        results = []
        for c in range(NCORES):
            m = {}
            for i, name in enumerate(out_names):
                per = out_arrs[i].shape[0] // NCORES
                m[name] = out_arrs[i][c * per : (c + 1) * per]
            results.append(m)
        return results

    def time_exec(in_maps, reps=10):
        import time as _time
        import jax.numpy as jnp
        from jax.sharding import NamedSharding

        shd = NamedSharding(mesh, PartitionSpec("core"))
        dev_in = [jax.device_put(a, shd) for a in _concat_params(in_maps)]
        gshapes = [
            ((NCORES * z.shape[0],) + z.shape[1:], z.dtype) for z in zero_outs
        ]
        zmaker = jax.jit(
            lambda: tuple(jnp.zeros(sh, dt) for sh, dt in gshapes),
            out_shardings=tuple(shd for _ in gshapes),
        )
        times = []
        for _ in range(reps):
            z = zmaker()
            jax.block_until_ready(z)
            jax.block_until_ready(dev_in)
            t0 = _time.perf_counter()
            out = jitted(*dev_in, *z)
            jax.block_until_ready(out)
            times.append(_time.perf_counter() - t0)
        return times

    runner.time_exec = time_exec
    _CACHED[key] = runner
    return runner


def _make_in_maps(x, w1, w2, w_sp):
    x = np.ascontiguousarray(x, np.float32)
    w1blk, w2blk, wconv, cmask, ident, bsel = _host_inputs(
        np.asarray(w1, np.float32), np.asarray(w2, np.float32),
        np.asarray(w_sp, np.float32),
    )
    try:
        import ml_dtypes
        bf16 = ml_dtypes.bfloat16
    except ImportError:
        import jax.numpy as jnp
        bf16 = jnp.bfloat16
    in_maps = []
    for core in range(NCORES):
        in_maps.append(
            {
                "xin": np.ascontiguousarray(x[:, :, core * DL : (core + 1) * DL]),
                "w1blk": w1blk,
                "w2blk": w2blk,
                "wconv": wconv.astype(bf16),
                "cmask": cmask.astype(bf16),
                "ident": ident,
                "bsel": bsel.astype(bf16),
                "selh": _selh_for_core(core).astype(bf16),
            }
        )
    return in_maps


def kernel(x, w1, w2, w_sp):
    in_maps = _make_in_maps(x, w1, w2, w_sp)
    runner = _get_runner()
    outs = runner(in_maps)
    return np.concatenate([outs[c]["y"] for c in range(NCORES)], axis=2)
